# revision 8
# baseline (speedup 1.0000x reference)
"""VQ-VAE NearestEmbedEMA forward+EMA-update kernel for 8 Trainium2 NeuronCores.

Strategy (data-parallel over batch):
  - Each of the 8 cores processes 16 of the 128 batch images (16384 tokens).
  - Per 128-token tile: PE computes scores = x@w - 0.5|w|^2 (argmax of this
    == argmin of L2 distance), DVE finds max+index, builds a one-hot, PE
    accumulates [embed_sum; counts] = [x;1]^T @ onehot into PSUM across all
    tiles, and PE reconstructs the quantized output via
    weightT-chunk @ transpose(onehot).
  - counts+embed_sum are AllReduce'd across the 8 cores, then every core
    computes the identical EMA/normalization updates.
"""

import sys

sys.path.insert(0, "/opt/trn_rl_repo")

import numpy as np

N_CORES = 8
B, D, H, W = 128, 64, 32, 32
HW = H * W
K = 512
B_LOC = B // N_CORES          # images per core
TPI = HW // 128               # 128-token tiles per image (8)
NT = B_LOC * TPI              # total tiles per core (128)

_CACHE = {}


def _build(n_cores):
    import concourse.bacc as bacc
    import concourse.mybir as mybir
    import concourse.tile as tile

    dt = mybir.dt
    f32 = dt.float32
    Alu = mybir.AluOpType
    Ax = mybir.AxisListType

    nc = bacc.Bacc(None, target_bir_lowering=False)

    x_in = nc.dram_tensor("x", [B_LOC, D, HW], f32, kind="ExternalInput")
    wfull_in = nc.dram_tensor("wfull", [D + 1, K], f32, kind="ExternalInput")
    wt_in = nc.dram_tensor("wt", [K, D], f32, kind="ExternalInput")
    iota_in = nc.dram_tensor("iota", [128, K], f32, kind="ExternalInput")
    id_in = nc.dram_tensor("id128", [128, 128], f32, kind="ExternalInput")
    cs_in = nc.dram_tensor("cluster_size", [1, K], f32, kind="ExternalInput")
    ea_in = nc.dram_tensor("embed_avg", [D, K], f32, kind="ExternalInput")
    pc_in = nc.dram_tensor("prev_cluster", [1, K], f32, kind="ExternalInput")

    res_out = nc.dram_tensor("result", [B_LOC, D, HW], f32, kind="ExternalOutput")
    am_out = nc.dram_tensor("argmin", [B_LOC, TPI, 128], dt.int32, kind="ExternalOutput")
    nw_out = nc.dram_tensor("new_weight", [D, K], f32, kind="ExternalOutput")
    ncs_out = nc.dram_tensor("new_cluster_size", [1, K], f32, kind="ExternalOutput")
    nea_out = nc.dram_tensor("new_embed_avg", [D, K], f32, kind="ExternalOutput")
    npc_out = nc.dram_tensor("new_prev_cluster", [1, K], f32, kind="ExternalOutput")

    with tile.TileContext(nc) as tc:
        with (
            tc.tile_pool(name="const", bufs=1) as cp,
            tc.tile_pool(name="xin", bufs=3) as xp,
            tc.tile_pool(name="sb", bufs=4) as sbp,
            tc.tile_pool(name="stage", bufs=2) as stp,
            tc.tile_pool(name="fin", bufs=1) as fp,
            tc.tile_pool(name="ps_s", bufs=2, space="PSUM") as ps_s,
            tc.tile_pool(name="ps_x", bufs=2, space="PSUM") as ps_x,
            tc.tile_pool(name="ps_t", bufs=1, space="PSUM") as ps_t,
            tc.tile_pool(name="ps_q", bufs=2, space="PSUM") as ps_q,
            tc.tile_pool(name="ps_e", bufs=1, space="PSUM") as ps_e,
            tc.tile_pool(name="dram", bufs=1, space="DRAM") as dramp,
        ):
            # ---- constants ----
            wfull = cp.tile([D + 1, K], f32, tag="wfull")
            nc.sync.dma_start(wfull[:], wfull_in[:])
            wt = []
            for c in range(4):
                t = cp.tile([128, D], f32, tag=f"wt{c}")
                nc.sync.dma_start(t[:], wt_in[128 * c : 128 * (c + 1), :])
                wt.append(t)
            iota = cp.tile([128, K], f32, tag="iota")
            nc.sync.dma_start(iota[:], iota_in[:])
            id128 = cp.tile([128, 128], f32, tag="id128")
            nc.sync.dma_start(id128[:], id_in[:])
            cs_sb = cp.tile([1, K], f32, tag="cs")
            nc.sync.dma_start(cs_sb[:], cs_in[:])
            ea_sb = cp.tile([D, K], f32, tag="ea")
            nc.sync.dma_start(ea_sb[:], ea_in[:])
            pc_sb = cp.tile([1, K], f32, tag="pc")
            nc.sync.dma_start(pc_sb[:], pc_in[:])
            ones64 = cp.tile([1, D], f32, tag="ones64")
            nc.vector.memset(ones64[:], 1.0)

            embed_ps = ps_e.tile([D + 1, K], f32, tag="embed")

            for b in range(B_LOC):
                x1 = xp.tile([D + 1, HW], f32, tag="x1")
                nc.sync.dma_start(x1[:D, :], x_in[b])
                nc.gpsimd.memset(x1[D : D + 1, :], 1.0)
                res_stage = stp.tile([D, HW], f32, tag="res")
                am_stage = stp.tile([128, TPI], f32, tag="am")
                for t in range(TPI):
                    gidx = b * TPI + t
                    lhs = x1[:, 128 * t : 128 * (t + 1)]
                    # scores [128 tok, 512 codes]
                    s_ps = ps_s.tile([128, K], f32, tag="s")
                    nc.tensor.matmul(s_ps[:], lhs, wfull[:], start=True, stop=True)
                    # x tile transposed (tokens on partitions), incl ones col
                    xT_ps = ps_x.tile([128, D + 1], f32, tag="xT")
                    nc.tensor.transpose(xT_ps[:], lhs, id128[: D + 1, : D + 1])
                    s_sb = sbp.tile([128, K], f32, tag="s_sb")
                    nc.scalar.copy(s_sb[:], s_ps[:])
                    xT_sb = sbp.tile([128, D + 1], f32, tag="xT_sb")
                    nc.scalar.copy(xT_sb[:], xT_ps[:])
                    # argmax over codes
                    m8 = sbp.tile([128, 8], f32, tag="m8")
                    nc.vector.max(m8[:], s_sb[:])
                    i8 = sbp.tile([128, 8], dt.uint32, tag="i8")
                    nc.vector.max_index(i8[:], m8[:], s_sb[:])
                    idxf = sbp.tile([128, 1], f32, tag="idxf")
                    nc.vector.tensor_copy(idxf[:], i8[:, 0:1])
                    nc.vector.tensor_copy(am_stage[:, t : t + 1], idxf[:])
                    onehot = sbp.tile([128, K], f32, tag="onehot")
                    nc.vector.tensor_scalar(
                        onehot[:], iota[:], scalar1=idxf[:], scalar2=None,
                        op0=Alu.is_equal,
                    )
                    # [embed_sum; counts] accumulation
                    nc.tensor.matmul(
                        embed_ps[:], xT_sb[:], onehot[:],
                        start=(gidx == 0), stop=(gidx == NT - 1),
                        skip_group_check=True,
                    )
                    # quantized output: wT-chunk @ onehot^T
                    ohT_ps = ps_t.tile([128, K], f32, tag="ohT")
                    for c in range(4):
                        nc.tensor.transpose(
                            ohT_ps[:, 128 * c : 128 * (c + 1)],
                            onehot[:, 128 * c : 128 * (c + 1)],
                            id128[:],
                        )
                    ohT_sb = sbp.tile([128, K], f32, tag="ohT_sb")
                    nc.scalar.copy(ohT_sb[:], ohT_ps[:])
                    q_ps = ps_q.tile([D, 128], f32, tag="q")
                    for c in range(4):
                        nc.tensor.matmul(
                            q_ps[:], wt[c][:], ohT_sb[:, 128 * c : 128 * (c + 1)],
                            start=(c == 0), stop=(c == 3), skip_group_check=True,
                        )
                    nc.scalar.copy(res_stage[:, 128 * t : 128 * (t + 1)], q_ps[:])
                # store image outputs
                nc.sync.dma_start(res_out[b], res_stage[:])
                amT_ps = ps_q.tile([TPI, 128], f32, tag="q")
                nc.tensor.transpose(amT_ps[:], am_stage[:], id128[:])
                am_i32 = sbp.tile([TPI, 128], dt.int32, tag="am_i32")
                nc.vector.tensor_copy(am_i32[:], amT_ps[:])
                nc.sync.dma_start(am_out[b], am_i32[:])

            # ---- collective: sum [embed_sum; counts] over cores ----
            esum_sb = fp.tile([D + 1, K], f32, tag="esum")
            nc.scalar.copy(esum_sb[:], embed_ps[:])
            cc_in = dramp.tile([D + 1, K], f32, tag="cc_in")
            cc_out = dramp.tile([D + 1, K], f32, tag="cc_out")
            nc.sync.dma_start(cc_in[:], esum_sb[:])
            nc.gpsimd.collective_compute(
                "AllReduce", Alu.add,
                replica_groups=[list(range(n_cores))],
                ins=[cc_in.opt()], outs=[cc_out.opt()],
            )
            esum_t = fp.tile([D, K], f32, tag="esum_t")
            nc.sync.dma_start(esum_t[:], cc_out[:D, :])
            counts_t = fp.tile([1, K], f32, tag="counts_t")
            nc.sync.dma_start(counts_t[:], cc_out[D : D + 1, :])
            counts = counts_t[:]
            esum = esum_t[:]

            # ---- EMA updates (identical on every core) ----
            npc_sb = fp.tile([1, K], f32, tag="npc")
            nc.vector.tensor_tensor(npc_sb[:], pc_sb[:], counts, Alu.add)
            nc.sync.dma_start(npc_out[:], npc_sb[:])

            eq0 = fp.tile([1, K], f32, tag="eq0")
            nc.vector.tensor_scalar(eq0[:], counts, scalar1=0.0, scalar2=None, op0=Alu.is_equal)
            ccnt = fp.tile([1, K], f32, tag="ccnt")
            nc.vector.tensor_tensor(ccnt[:], counts, eq0[:], Alu.add)

            a1 = fp.tile([1, K], f32, tag="a1")
            nc.vector.tensor_scalar(a1[:], cs_sb[:], scalar1=0.99, scalar2=None, op0=Alu.mult)
            a2 = fp.tile([1, K], f32, tag="a2")
            nc.vector.tensor_scalar(a2[:], ccnt[:], scalar1=0.01, scalar2=None, op0=Alu.mult)
            ncs_sb = fp.tile([1, K], f32, tag="ncs")
            nc.vector.tensor_tensor(ncs_sb[:], a1[:], a2[:], Alu.add)
            nc.sync.dma_start(ncs_out[:], ncs_sb[:])

            e1 = fp.tile([D, K], f32, tag="e1")
            nc.vector.tensor_scalar(e1[:], ea_sb[:], scalar1=0.99, scalar2=None, op0=Alu.mult)
            e2 = fp.tile([D, K], f32, tag="e2")
            nc.vector.tensor_scalar(e2[:], esum, scalar1=0.01, scalar2=None, op0=Alu.mult)
            nea_sb = fp.tile([D, K], f32, tag="nea")
            nc.vector.tensor_tensor(nea_sb[:], e1[:], e2[:], Alu.add)
            nc.sync.dma_start(nea_out[:], nea_sb[:])

            n_sb = fp.tile([1, 1], f32, tag="n")
            nc.vector.reduce_sum(n_sb[:], ncs_sb[:], axis=Ax.X)
            nd = fp.tile([1, 1], f32, tag="nd")
            nc.vector.tensor_scalar(nd[:], n_sb[:], scalar1=K * 1e-5, scalar2=None, op0=Alu.add)
            ndi = fp.tile([1, 1], f32, tag="ndi")
            nc.vector.reciprocal(ndi[:], nd[:])
            af = fp.tile([1, 1], f32, tag="af")
            nc.vector.tensor_tensor(af[:], n_sb[:], ndi[:], Alu.mult)
            csn = fp.tile([1, K], f32, tag="csn")
            nc.vector.tensor_scalar(
                csn[:], ncs_sb[:], scalar1=1e-5, scalar2=af[:], op0=Alu.add, op1=Alu.mult
            )
            # broadcast csn across 64 partitions via PE, then divide
            csb_ps = ps_x.tile([D, K], f32, tag="xT")
            nc.tensor.matmul(csb_ps[:], ones64[:], csn[:], start=True, stop=True)
            csb_sb = fp.tile([D, K], f32, tag="csb")
            nc.scalar.copy(csb_sb[:], csb_ps[:])
            cinv = fp.tile([D, K], f32, tag="cinv")
            nc.vector.reciprocal(cinv[:], csb_sb[:])
            nw_sb = fp.tile([D, K], f32, tag="nw")
            nc.vector.tensor_tensor(nw_sb[:], nea_sb[:], cinv[:], Alu.mult)
            nc.sync.dma_start(nw_out[:], nw_sb[:])

    nc.finalize()
    return nc


def _get_nc(n_cores=N_CORES):
    if n_cores not in _CACHE:
        _CACHE[n_cores] = _build(n_cores)
    return _CACHE[n_cores]


LAST_EXEC_NS = None
LAST_RES = None


def make_timed_runner(in_maps):
    """Build a reusable jitted SPMD executor (mirrors bass2jax.run_bass_via_pjrt
    multi-core path) so repeated executions can be wall-clock timed without
    per-call retracing."""
    import jax
    import numpy as np
    from jax.sharding import Mesh, PartitionSpec
    from jax.experimental.shard_map import shard_map
    import concourse.bass2jax as b2j
    import concourse.mybir as mybir

    nc = _get_nc()
    b2j.install_neuronx_cc_hook()
    partition_name = nc.partition_id_tensor.name if nc.partition_id_tensor else None
    in_names, out_names, out_avals, zero_outs = [], [], [], []
    for alloc in nc.m.functions[0].allocations:
        if not isinstance(alloc, mybir.MemoryLocationSet):
            continue
        name = alloc.memorylocations[0].name
        if alloc.kind == "ExternalInput":
            if name != partition_name:
                in_names.append(name)
        elif alloc.kind == "ExternalOutput":
            out_names.append(name)
            shape = tuple(alloc.tensor_shape)
            dtype = mybir.dt.np(alloc.dtype)
            out_avals.append(jax.core.ShapedArray(shape, dtype))
            zero_outs.append(np.zeros(shape, dtype))
    n_params = len(in_names)
    n_outs = len(out_avals)
    all_in_names = list(in_names) + list(out_names)
    if partition_name is not None:
        all_in_names.append(partition_name)

    def _body(*args):
        operands = list(args)
        if partition_name is not None:
            operands.append(b2j.partition_id_tensor())
        outs = b2j._bass_exec_p.bind(
            *operands,
            out_avals=tuple(out_avals),
            in_names=tuple(all_in_names),
            out_names=tuple(out_names),
            lowering_input_output_aliases=(),
            sim_require_finite=True,
            sim_require_nnan=True,
            nc=nc,
        )
        return tuple(outs)

    devices = jax.devices()[:N_CORES]
    mesh = Mesh(np.asarray(devices), ("core",))
    in_specs = (PartitionSpec("core"),) * (n_params + n_outs)
    out_specs = (PartitionSpec("core"),) * n_outs
    sharded = jax.jit(
        shard_map(_body, mesh=mesh, in_specs=in_specs, out_specs=out_specs, check_rep=False),
        keep_unused=True,
    )
    per_core = [[np.asarray(m[name]) for name in in_names] for m in in_maps]
    concat_in = [
        np.concatenate([per_core[c][i] for c in range(N_CORES)], axis=0)
        for i in range(n_params)
    ] + [np.concatenate([z] * N_CORES, axis=0) for z in zero_outs]
    concat_dev = [jax.device_put(a) for a in concat_in]

    def run():
        outs = sharded(*concat_dev)
        jax.block_until_ready(outs)
        return outs

    return run


def prep_in_maps(x, weight, cluster_size, embed_avg, prev_cluster):
    x = np.ascontiguousarray(np.asarray(x, np.float32))
    weight = np.ascontiguousarray(np.asarray(weight, np.float32))
    cluster_size = np.asarray(cluster_size, np.float32)
    embed_avg = np.ascontiguousarray(np.asarray(embed_avg, np.float32))
    prev_cluster = np.asarray(prev_cluster, np.float32)

    wfull = np.concatenate(
        [weight, (-0.5 * np.sum(weight * weight, axis=0, dtype=np.float32))[None]], 0
    ).astype(np.float32)
    wt = np.ascontiguousarray(weight.T)
    iota = np.ascontiguousarray(
        np.broadcast_to(np.arange(K, dtype=np.float32), (128, K))
    )
    id128 = np.eye(128, dtype=np.float32)
    xr = x.reshape(B, D, HW)

    in_maps = []
    for c in range(N_CORES):
        in_maps.append(
            {
                "x": np.ascontiguousarray(xr[B_LOC * c : B_LOC * (c + 1)]),
                "wfull": wfull,
                "wt": wt,
                "iota": iota,
                "id128": id128,
                "cluster_size": cluster_size.reshape(1, K),
                "embed_avg": embed_avg,
                "prev_cluster": prev_cluster.reshape(1, K),
            }
        )
    return in_maps


def kernel(x, weight, cluster_size, embed_avg, prev_cluster):
    global LAST_EXEC_NS, LAST_RES
    from concourse.bass_utils import run_bass_kernel_spmd

    in_maps = prep_in_maps(x, weight, cluster_size, embed_avg, prev_cluster)
    nc = _get_nc()
    res = run_bass_kernel_spmd(nc, in_maps, list(range(N_CORES)))
    LAST_EXEC_NS = res.exec_time_ns
    LAST_RES = res
    rs = res.results
    result = np.concatenate([rs[c]["result"] for c in range(N_CORES)], 0).reshape(
        B, D, H, W
    )
    argmin = np.concatenate([rs[c]["argmin"] for c in range(N_CORES)], 0).reshape(
        B, H, W
    ).astype(np.int32)
    new_weight = rs[0]["new_weight"]
    new_cluster_size = rs[0]["new_cluster_size"].reshape(K)
    new_embed_avg = rs[0]["new_embed_avg"]
    new_prev_cluster = rs[0]["new_prev_cluster"].reshape(K)
    return (result, argmin, new_weight, new_cluster_size, new_embed_avg, new_prev_cluster)
